# revision 29
# baseline (speedup 1.0000x reference)
"""Bass/Trainium2 kernel for DegreeOnlyFiltration (segment max + gather-divide).

Contract: kernel(**inputs) takes FULL inputs (node_deg [N] f32, sample_pos
[G+1] i32 CSR boundaries) and returns the FULL output node_deg / seg_max.

Strategy (per the sharding hint): segments are contiguous; the expected input
has uniform boundaries (sample_pos = arange(G+1) * W).  We shard node_deg by
whole segments across the 8 NeuronCores (pure data parallel, no cross-core
traffic).  On each core the shard is [512, 4096] f32 (8 MiB), processed as 4
chunks of [128 segments x 4096] (2 MiB, one segment per partition row, 16 KB
DMA descriptors -> 16 KB packets at ~28 GB/s per SDMA engine).  Engine roles:
  Sync (SP HWDGE ring):  all 4 input doorbells up front, then the held
                         output doorbells (chunks 2-3), whose descriptors
                         queue FIFO behind the remaining inputs and so start
                         draining exactly when the reads finish.
  DVE:                   reduce_max (1x-mode-capped, 4.4 us/chunk) +
                         reciprocal + 41/64 of the multiply (2x fp32
                         tensor_scalar), in-place.
  ACT (ACT HWDGE ring):  23/64 of the multiply (activation-scale), then the
                         EARLY output doorbells (chunks 0-1) -- early writes
                         fill the ~18% SDMA idle of the HBM-read-bound
                         phase; a tiny warmup write first absorbs the ACT
                         ring's ~4 us cold-start latency.
Measured (n=20+): ~53 us in the common mode, ~60 us when SDMA engine 15
(which also serves both rings' descriptor fetches and profiling traffic)
carries a few extra us of background load; baseline was 65.5/59.6 us.
"""

import os

import numpy as np

import concourse.bacc as bacc
import concourse.mybir as mybir
import concourse.tile as tile
from concourse.bass_utils import run_bass_kernel_spmd

N_CORES = 8
P = 128  # SBUF partitions

# Populated after each traced run (test harness reads these).
LAST_EXEC_TIME_NS = None
LAST_RESULTS = None

_NC_CACHE = {}


def _build_uniform_nc(segs_per_core: int, width: int):
    """SPMD program: x [segs_per_core, width] f32 -> y = x / rowmax(x)."""
    assert segs_per_core % P == 0
    n_chunks = segs_per_core // P
    f32 = mybir.dt.float32

    nc = bacc.Bacc("TRN2", target_bir_lowering=False, debug=False,
                   num_devices=N_CORES, enable_partition_id=False,
                   enable_asserts=False)
    x = nc.dram_tensor("x", [segs_per_core, width], f32, kind="ExternalInput")
    y = nc.dram_tensor("y", [segs_per_core, width], f32, kind="ExternalOutput")

    # NOTE: tensor_tensor_reduce (pairwise-max of the column-halves with a
    # max accumulator) would halve the 4.4 us reduce on the critical tail
    # chain, but it wedges the exec unit in this runtime
    # (NRT_EXEC_UNIT_UNRECOVERABLE) -- even the known-good qr.py pattern.
    # tensor_reduce is capped at 1x mode (no 2x/4x uops), so 4.4 us/chunk
    # is the floor.
    # Multiply halves run concurrently on DVE (2x mode, ~0.61 ns/col) and
    # ACT (~0.98 ns/col + fixed); they balance at about 64/36.
    wd = (41 * width // 64) if width % 64 == 0 else 0

    # Release the first two chunks' outputs early on the ACT ring; hold the
    # rest poised on the SP ring behind the inputs.  (Benchmarked n_early in
    # {0,1,2,3}: total time is flat within noise -- early release fills
    # read-phase engine idle but delays the last chunk's landing ~1:1.)
    n_early = int(os.environ.get("KERNEL_N_EARLY", "2"))

    with tile.TileContext(nc) as tc:
        with tc.tile_pool(name="p", bufs=1) as pool:
            # All input DMAs up front on the SP HWDGE ring (Sync engine does
            # nothing else, so every doorbell rings back-to-back).  Single
            # full-width DMAs keep every descriptor at 16 KB: smaller input
            # packets would round-robin 1:1 against 16 KB output packets at
            # the SDMA engines and the input tail would crawl.
            # (A tiny SP-ring read-warmup before the inputs was benchmarked:
            # it STALLS the ring -- sub-512B DMAs read-modify-write -- and
            # pushed chunk 0's first packet from 8.7 us to 11.5 us.  The SP
            # ring needs no warmup; its first real DMA ramps in ~1.5 us.)
            tins = []
            for t in range(n_chunks):
                tin = pool.tile([P, width], f32, tag=f"tin{t}")
                # (single_packet=True here starves the early writes' idle-
                # fill -- benchmarked worse; keep inputs multi-packet.)
                nc.sync.dma_start(tin[:], x[t * P:(t + 1) * P, :])
                tins.append(tin)

            # Warm up the ACT HWDGE ring with a small throwaway write (the
            # real out0 overwrites it, FIFO-later on the same ring): the
            # ring's first DMA otherwise pays ~4 us of cold doorbell-to-
            # first-packet latency right when early writes should be
            # filling the read phase's idle engine slots.  Must be >=512 B:
            # smaller writes take the read-modify-write path (fragmented
            # 8-byte packets, ~4.4 us to land, extra HBM reads).
            wn = min(512, width)
            warm = pool.tile([1, wn], f32, tag="warm")
            nc.gpsimd.memset(warm[:], 0.0)
            nc.scalar.dma_start(y[0:1, 0:wn], warm[:])

            for t in range(n_chunks):
                tin = tins[t]
                m = pool.tile([P, 1], f32, tag=f"m{t}")
                nc.vector.reduce_max(m[:], tin[:],
                                     axis=mybir.AxisListType.X)
                r = pool.tile([P, 1], f32, tag=f"r{t}")
                nc.vector.reciprocal(r[:], m[:])
                if wd:
                    nc.vector.tensor_scalar_mul(tin[:, :wd], tin[:, :wd],
                                                r[:])
                    nc.scalar.mul(tin[:, wd:], tin[:, wd:], r[:])
                else:
                    nc.scalar.mul(tin[:], tin[:], r[:])
                # Output pacing: every output byte drained before the last
                # chunk lands steals read bandwidth 1:1 (per-packet round-
                # robin between the two HWDGE rings), delaying the tail
                # chain (last-land -> reduce -> mul -> doorbell -> drain).
                # But the read phase leaves the SDMA engines ~18% idle
                # (HBM-read-bound), which early writes fill for free.  So:
                # release the first n_early chunks' outputs on the ACT ring;
                # hold the rest POISED on the SP ring, where FIFO order
                # behind the remaining input descriptors releases them the
                # moment the reads finish -- full-rate writes, zero read
                # stealing, no idle gap.
                if t < n_early:
                    with tc.high_priority():
                        nc.scalar.dma_start(y[t * P:(t + 1) * P, :], tin[:])
                else:
                    # single_packet: the held outputs drain with no competing
                    # stream, so packing each engine's descriptors into one
                    # packet costs no interleave and trims per-packet
                    # boundaries/notify events on the tail drain.
                    nc.sync.dma_start(y[t * P:(t + 1) * P, :], tin[:],
                                      single_packet=True)
    nc.compile()
    return nc


def _uniform_width(sample_pos: np.ndarray, n: int):
    """Return segment width W if boundaries are uniform (pos = arange*W)."""
    if sample_pos[0] != 0 or sample_pos[-1] != n:
        return None
    diffs = np.diff(sample_pos)
    if diffs.size == 0 or np.any(diffs != diffs[0]):
        return None
    return int(diffs[0])


def _host_fallback(node_deg: np.ndarray, sample_pos: np.ndarray) -> np.ndarray:
    """Exact mirror of the reference semantics for non-uniform boundaries."""
    import jax

    with jax.default_device(jax.devices("cpu")[0]):
        import jax.numpy as jnp

        deg = jnp.asarray(node_deg)
        pos = jnp.asarray(sample_pos)
        n = deg.shape[0]
        g = pos.shape[0] - 1
        seg_ids = jnp.searchsorted(pos[1:], jnp.arange(n, dtype=pos.dtype),
                                   side="right")
        seg_max = jax.ops.segment_max(deg, seg_ids, num_segments=g)
        return np.asarray(deg / seg_max[seg_ids])


def kernel(node_deg: np.ndarray, sample_pos: np.ndarray) -> np.ndarray:
    global LAST_EXEC_TIME_NS, LAST_RESULTS

    node_deg = np.asarray(node_deg, dtype=np.float32)
    sample_pos = np.asarray(sample_pos, dtype=np.int32)
    n = node_deg.shape[0]
    g = sample_pos.shape[0] - 1

    width = _uniform_width(sample_pos, n)
    if (width is None or width < 512 or g % N_CORES != 0
            or (g // N_CORES) % P != 0):
        return _host_fallback(node_deg, sample_pos)

    segs_per_core = g // N_CORES

    key = (segs_per_core, width, os.environ.get("KERNEL_N_EARLY", "2"))
    if key not in _NC_CACHE:
        _NC_CACHE[key] = _build_uniform_nc(segs_per_core, width)
    nc = _NC_CACHE[key]

    shards = node_deg.reshape(N_CORES, segs_per_core, width)
    in_maps = [{"x": shards[c]} for c in range(N_CORES)]

    trace = bool(int(os.environ.get("KERNEL_TRACE", "0")))
    try:
        res = run_bass_kernel_spmd(nc, in_maps, core_ids=list(range(N_CORES)),
                                   trace=trace)
    except Exception:
        if not trace:
            raise
        # Trace post-processing can fail in sandboxes; results still matter.
        res = run_bass_kernel_spmd(nc, in_maps, core_ids=list(range(N_CORES)),
                                   trace=False)
    LAST_EXEC_TIME_NS = res.exec_time_ns
    LAST_RESULTS = res
    out = np.concatenate([res.results[c]["y"].reshape(-1)
                          for c in range(N_CORES)])
    return out.astype(np.float32, copy=False)
